# revision 2
# baseline (speedup 1.0000x reference)
"""Trainium2 Bass kernel for nn_BinarizedConv2d (3-bit-packed weight stream).

Math: activation[d, o] = sum_k weight_noise[d, o, k] * x[d, k]
      out[d, o]        = activation[d, o] > bias_noise[d, o]
with D=128 directions, O=256 out channels, K=2304 reduction length.
Sharding: D split across 8 NeuronCores (16 directions per core), no
collectives.

Weights and x are 0/1 bits, so THREE adjacent k-bits are packed host-side
into one fp8 byte as single-bit fields of the e4m3 ENCODING:
    enc = w0*0x40 | w1*0x20 | w2*0x08
Because each field is a single bit, (enc & mask) is always a valid fp8
float with an exact per-bit value:
    enc & 0x40 = 2.0   * w0     (exponent bit)
    enc & 0x20 = 0.125 * w1     (exponent bit)
    enc & 0x08 = 2^-6  * w2     (mantissa msb)
so three uint16-bitcast AND ops (DVE 4x perf mode) reconstruct three exact
operand streams from a 3.15 MB/core HBM stream (3 bits/byte; the kernel is
HBM-bound). The matvec is three accumulating matmul streams per direction
with host-prescaled x coefficients 0.5*x0 / 8*x1 / 64*x2 (exact fp8), so
every partial product is 0 or 1 and fp32 PSUM accumulation is exact.

The threshold is folded into PSUM by one tiny fp16 matmul per quad
(stationary selneg[j, m] = -1 iff m//32 == j, moving operand the per-quad
row of kf = floor(bias), integers ~576, exact in fp16), run FIRST with
start=True - floor(bias) is an integer so all partials stay exact. For
integer activations,  act > bias <=> act - floor(bias) > 0.5,  so the
epilogue is a single-src (psum is_gt 0.5) -> uint8 on DVE.

Scheduling (from trace analysis of the fp8/b=2 versions): ~0.7us issue
cost per dma_start and ~8 HWDGE completion semaphores; big chunks sustain
~430 GB/s where many small ones starve (~350); each chunk's completion
semaphore fires 2-4us after its bytes land (HBM receipt round-trip), so
the last chunk is kept small and the last quad's matmuls are tile-
interleaved; ~30 dummy matmuls into an unused PSUM window pre-warm the PE
clock gate (HAM lifts 1.2->2.4 GHz after ~3.4us of sustained activity);
bulk results fly out early and only 1KB rides the final DMA receipt.
"""

import numpy as np
import ml_dtypes

D = 128          # directions (ES population)
O = 256          # out channels
K = 2304         # flattened reduction length
NT = 6           # packed k-tiles of 128 (K/3 = 768 triples)
P = 128          # partitions
NCORES = 8
DPC = D // NCORES  # directions per core
NQ = DPC // 4      # quads per core
NS = 3             # bit-streams per packed byte

FP8 = ml_dtypes.float8_e4m3
MASKS = (0x4040, 0x2020, 0x0808)
SCALES = (0.5, 8.0, 64.0)   # coefficient prescale per stream (host side)

_nc_cache = {}

# weight chunk schedule: (quad, tile0, tile1) in consume order
CHUNKS = [
    (0, 0, 2), (0, 2, 4), (0, 4, 6),
    (1, 0, 2), (1, 2, 4), (1, 4, 6),
    (2, 0, 2), (2, 2, 4), (2, 4, 6),
    (3, 0, 2), (3, 2, 4), (3, 4, 6),
]
RING_OF = [0, 0, 0, 0, 1, 0, 1, 0, 1, 0, 1, 0]


def _emit(tc, res_ap, wT_ap, xT_ap, hdr_ap):
    """Emit the per-core program into TileContext tc."""
    import concourse.mybir as mybir

    nc = tc.nc
    fp8 = mybir.dt.float8e4
    u16 = mybir.dt.uint16
    f16 = mybir.dt.float16
    f32 = mybir.dt.float32
    u8 = mybir.dt.uint8
    XN = DPC * NT  # 96 coefficient columns per stream

    with (
        tc.tile_pool(name="w", bufs=1) as wp,
        tc.tile_pool(name="small", bufs=1) as sp,
        tc.tile_pool(name="act", bufs=1) as ap_pool,
        tc.tile_pool(name="ps", bufs=1, space="PSUM") as pp,
    ):
        # prescaled x coefficient streams, first on the SP ring:
        # xc[:, s*XN + d*NT + t] = SCALES[s] * x[d0+d, 3*(t*128+p) + s]
        xc = sp.tile([P, NS * XN], fp8)
        nc.sync.dma_start(out=xc[:], in_=xT_ap)
        # header on the ACT ring: kf = floor(bias) [4, NQ*O] ++ selneg [4,128]
        hdr = sp.tile([4, NQ * O + P], f16)
        nc.scalar.dma_start(out=hdr[:], in_=hdr_ap)

        ring = [nc.sync, nc.scalar]
        p_tiles = [wp.tile([P, NT * 4 * O], fp8, tag=f"p{q}", name=f"p_t{q}")
                   for q in range(NQ)]
        s_tiles = []
        for s in range(NS):
            row = []
            for q in range(NQ):
                t_ = wp.tile([P, NT * 4 * O], fp8, tag=f"s{s}q{q}",
                             name=f"s_t{s}_{q}")
                row.append(t_)
            s_tiles.append(row)
        for ci, (qi, t0, t1) in enumerate(CHUNKS):
            c0, c1 = t0 * 4 * O, t1 * 4 * O
            ring[RING_OF[ci]].dma_start(
                out=p_tiles[qi][:, c0:c1], in_=wT_ap[qi][:, c0:c1]
            )

        res_all = ap_pool.tile([P, NQ * O], u8)
        ps_all = pp.tile([P, 8 * 2 * O], f32)
        probe = sp.tile([1, 4], f32)

        # PE warm-up (HAM clock gate): ~3.5us of dummy matmuls into an
        # unused PSUM window before the first weight chunk lands.
        scratch = sp.tile([P, 2 * O], fp8)
        nc.vector.memset(scratch[:], 0.0)
        neg128 = sp.tile([P, 1], f32)
        nc.vector.memset(neg128[:], -128.0)
        for w in range(18):
            nc.tensor.matmul(
                ps_all[0:32, O : 2 * O],
                scratch[:, 0:32],
                scratch[:, 0:O],
                start=True,
                stop=True,
                tile_position=(0, 0),
                skip_group_check=True,
            )

        def derive(qi, t0, t1):
            c0, c1 = t0 * 4 * O, t1 * 4 * O
            for s in range(NS):
                nc.vector.tensor_scalar(
                    out=s_tiles[s][qi][:, c0:c1].bitcast(u16),
                    in0=p_tiles[qi][:, c0:c1].bitcast(u16),
                    scalar1=MASKS[s], scalar2=None,
                    op0=mybir.AluOpType.bitwise_and,
                )

        def mm_quad(q):
            win = slice(q * 2 * O, q * 2 * O + O)
            # -floor(bias) seeds the accumulation (integer => all partials
            # stay exact fp32 integers); keeping it FIRST removes it from
            # the kernel tail.
            nc.tensor.matmul(
                ps_all[:, win],
                hdr[0:4, NQ * O : NQ * O + P],
                hdr[0:4, q * O : (q + 1) * O],
                start=True,
                stop=False,
                skip_group_check=True,
            )
            # quads 0-2: stream-major (their derives finish mid-stream);
            # quad 3: tile-major so only the final small chunk's matmuls
            # trail the last DMA completion.
            if q == 3:
                order = [(t, s) for t in range(NT) for s in range(NS)]
            else:
                order = [(t, s) for s in range(NS) for t in range(NT)]
            for i, (t, s) in enumerate(order):
                for j in range(4):
                    d = q * 4 + j
                    nc.tensor.matmul(
                        ps_all[32 * j : 32 * (j + 1), win],
                        xc[:, s * XN + d * NT + t : s * XN + d * NT + t + 1]
                        .broadcast_to((P, 32)),
                        s_tiles[s][q][:, (t * 4 + j) * O : (t * 4 + j + 1) * O],
                        start=False,
                        stop=(i == len(order) - 1 and j == 3),
                        tile_position=(0, 32 * j),
                        skip_group_check=True,
                    )

        # Compares on the (otherwise idle) ACT engine so the DVE FIFO is a
        # pure derive chain (the derive chain is the measured critical path
        # from the first chunk semaphore to the last matmul). For integer
        # ps, act > bias <=> ps >= 1, and sigmoid(256*ps - 128) saturates
        # to exactly 1.0 / 0.0, so the uint8 cast is the exact predicate.
        # The probe triggers the one-time ~2.7us sigmoid table load early,
        # after the ACT-ring dma_starts.
        nc.scalar.activation(
            out=probe[:], in_=ps_all[0:1, O : O + 4],
            func=mybir.ActivationFunctionType.Sigmoid, scale=256.0,
            bias=neg128[0:1, :],
        )

        def compare(q):
            nc.scalar.activation(
                out=res_all[:, q * O : (q + 1) * O],
                in_=ps_all[:, q * 2 * O : q * 2 * O + O],
                func=mybir.ActivationFunctionType.Sigmoid,
                scale=256.0, bias=neg128[:],
            )

        derive(0, 0, 2)
        derive(0, 2, 4)
        derive(0, 4, 6)
        mm_quad(0)
        derive(1, 0, 2)
        derive(1, 2, 4)
        derive(1, 4, 6)
        compare(0)
        mm_quad(1)
        derive(2, 0, 2)
        derive(2, 2, 4)
        derive(2, 4, 6)
        compare(1)
        mm_quad(2)
        derive(3, 0, 2)
        derive(3, 2, 4)
        derive(3, 4, 6)
        compare(2)
        mm_quad(3)
        # bulk of the results flies out while quad 3 still computes; only
        # the last quad's 1KB rides the final DMA receipt
        nc.scalar.dma_start(out=res_ap[:, : 3 * O], in_=res_all[0:P:32, : 3 * O])
        compare(3)
        nc.scalar.dma_start(out=res_ap[:, 3 * O :], in_=res_all[0:P:32, 3 * O :])


def _build():
    """Build the per-core Bass program (same NEFF on all 8 cores)."""
    import concourse.bacc as bacc
    import concourse.mybir as mybir
    from concourse.tile import TileContext

    nc = bacc.Bacc("TRN2", debug=False, enable_asserts=False)

    fp8 = mybir.dt.float8e4
    f16 = mybir.dt.float16
    u8 = mybir.dt.uint8

    # wT[q, p, (t*4 + j)*O + o] = packed triple for direction d0+4q+j,
    # triple index m = t*128 + p, byte w[3m]*0x40 | w[3m+1]*0x20 | w[3m+2]*0x08
    wT = nc.dram_tensor("wT", [NQ, P, NT * 4 * O], fp8, kind="ExternalInput")
    # xT[p, s*96 + d*6 + t] = SCALES[s] * x[d0+d, 3*(t*128+p) + s]
    xT = nc.dram_tensor("xT", [P, NS * DPC * NT], fp8, kind="ExternalInput")
    # hdr[j, q*O + o] = floor(bias_noise[d0+4q+j, o]); hdr[j, NQ*O + m] =
    # -1.0 if m//32 == j else 0 (the bias-fold selector)
    hdr = nc.dram_tensor("hdr", [4, NQ * O + P], f16, kind="ExternalInput")
    # res[j, q*O + o] = out[d0+4q+j, o]
    res = nc.dram_tensor("res", [4, NQ * O], u8, kind="ExternalOutput")

    with TileContext(nc) as tc:
        _emit(tc, res.ap(), wT.ap(), xT.ap(), hdr.ap())
    nc.compile()
    return nc


def prepare_inputs(weight_noise, bias_noise, x):
    """Host-side dtype cast + bit packing + layout transform + sharding.

    All transforms are data-independent (fixed index shuffles, the exact
    0/1 bit pack into encoding fields, fixed scalar prescale of the 0/1
    x-bits, and floor() of the compare constant); the reduction/compare
    math runs on device.
    """
    w = np.asarray(weight_noise)                      # [D, O, K] 0/1 floats
    wtri = w.reshape(D, O, K // 3, 3).astype(np.uint8)
    enc = (wtri[..., 0] * 0x40 + wtri[..., 1] * 0x20 + wtri[..., 2] * 0x08
           ).astype(np.uint8).view(FP8)               # [D, O, 768]
    # [D, O, NT, P] -> [D, P, NT, O]
    pT = np.ascontiguousarray(enc.reshape(D, O, NT, P).transpose(0, 3, 2, 1))

    xb = np.asarray(x).astype(np.float32)             # [D, K] 0/1
    xs = []
    for s in range(NS):
        v = (xb[:, s::3] * SCALES[s]).astype(FP8).reshape(D, NT, P)
        xs.append(np.ascontiguousarray(v.transpose(2, 0, 1)))  # [P, D, NT]

    kf = np.floor(np.asarray(bias_noise).astype(np.float64)).astype(np.float16)
    selneg = np.zeros((4, P), dtype=np.float16)
    for j in range(4):
        selneg[j, 32 * j : 32 * (j + 1)] = -1.0

    in_maps = []
    for c in range(NCORES):
        sl = slice(c * DPC, (c + 1) * DPC)
        # [d, p, t, o] -> [q, j, p, t, o] -> [q, p, t, j, o]
        wc = (
            pT[sl]
            .reshape(NQ, 4, P, NT, O)
            .transpose(0, 2, 3, 1, 4)
            .reshape(NQ, P, NT * 4 * O)
        )
        xcs = np.concatenate(
            [xs[s][:, sl, :].reshape(P, DPC * NT) for s in range(NS)], axis=1
        )
        kc = (
            kf[sl]
            .reshape(NQ, 4, O)
            .transpose(1, 0, 2)
            .reshape(4, NQ * O)
        )
        hc = np.concatenate([kc, selneg], axis=1)
        in_maps.append(
            {
                "wT": np.ascontiguousarray(wc),
                "xT": np.ascontiguousarray(xcs),
                "hdr": np.ascontiguousarray(hc),
            }
        )
    return in_maps


def run(weight_noise, bias_noise, x, trace=False, **spmd_kwargs):
    """Run on the 8 NeuronCores; returns (bool [D, O] output, results)."""
    from concourse.bass_utils import run_bass_kernel_spmd

    in_maps = prepare_inputs(weight_noise, bias_noise, x)
    if "nc" in _nc_cache:
        nc = _nc_cache["nc"]
    else:
        nc = _nc_cache["nc"] = _build()
    r = run_bass_kernel_spmd(
        nc, in_maps, core_ids=list(range(NCORES)), trace=trace, **spmd_kwargs
    )
    out = np.concatenate(
        [
            r.results[c]["res"]
            .reshape(4, NQ, O)
            .transpose(1, 0, 2)
            .reshape(DPC, O)
            for c in range(NCORES)
        ],
        axis=0,
    )
    return out.astype(bool), r


def core_output(res):
    """Decode one core's raw res buffer into [DPC, O] bool."""
    return (
        np.asarray(res).reshape(4, NQ, O).transpose(1, 0, 2).reshape(DPC, O)
    ).astype(bool)


def kernel(weight_noise, bias_noise, x):
    out, _ = run(weight_noise, bias_noise, x)
    return out



# revision 3
# speedup vs baseline: 1.1447x; 1.1447x over previous
"""Trainium2 Bass kernel for nn_BinarizedConv2d — v2 (full-density packing).

Math: activation[d, o] = sum_k weight_noise[d, o, k] * x[d, k]
      out[d, o]        = activation[d, o] > bias_noise[d, o]
with D=128 directions, O=256 out channels, K=2304 reduction length.
Sharding: D split across 8 NeuronCores (16 directions per core), no
collectives.

v2 changes vs the 3-bit/fp8 baseline (30.45us -> ~26.5us):
 1. FULL-DENSITY weight stream: 16 w-bits per u16 word (1.18 MB/core,
    3x less HBM than the 3-bit packing), so the DMA never stalls the
    unpack chain. The DVE unpack is 8 pass-types per packed region:
    (P >> s) & 0x4040 for s in {0,1} and (P << s) & 0x4040 for
    s in {1..6}; each pass extracts u16-bit pairs (6+s, 14+s) /
    (6-s, 14-s) into the e4m3 0x40 field of two adjacent output bytes
    (exact value 2.0 * bit). 8 passes cover all 16 bits. The pass map
    is element-local (out u16 elem w <- in u16 elem w), so any column
    sub-range derives independently. The DVE chain (24 ops, ~11.4us) is
    the critical path; tensor_scalar 2-op (shift+AND) costs the same as
    1-op AND, and no other engine supports bitwise ops (GpSimd: only
    add/mul-int32/power; ACT: only add/mul/sub — both verified against
    the walrus ISA checks), so full-density packing is derive-optimal.
 2. Per-bit x coefficients are uniformly 0.5*x (exact fp8); every
    partial product is exactly 1.0 or 0.0 in fp32 PSUM. The matmul
    stream keeps the baseline's 4-band column tiling (tile_position
    (0,32j)) which measures ~34ns/matmul pipelined — DoubleRow fp8
    cannot compose with column tiling (dst partition must be 0 per
    s3d3_mm_valid_dst_partition) and gives no net PE gain.
 3. Quads 1+2 fuse into single derive ops over a shared tile whose
    packed words / pass regions are COLUMN-CONCATENATED (plain 2D APs;
    3-D strided APs trip bounding-interval WAR tracking against the
    matmul reads and serialize the pipeline). Quad 3 uses an unequal
    region split 7x10+2 blocks so the last derive op (223ns) gates only
    2 matmuls + compare.
 4. Each quad accumulates into its OWN PSUM tile (a shared tile makes
    later quads' matmuls inherit false WAR deps on earlier compares,
    costing ~1.5us stalls each).
 5. Threshold fold via fp16 selneg matmul (start=True) + sigmoid-compare
    epilogue on ACT; results fly out in a 3-quad DMA + 1KB tail DMA.
 6. 18 dummy matmuls + early DMA issues ramp the HAM clock gate; a cold
    chip runs every engine ~1.2x slower and adds ~2us of DMA latency.

Derived layout per quad: DT [128, 72 blocks x 256] fp8, block B:
direction j = B & 3, k-tile t = B >> 2,
byte (p, B, o) = 2.0 * w[4q+j, o, 128*t + p]. Pass region pi covers
REG_LEN[q] blocks and is produced by one shift/AND over the packed
quad words (pass map is element-local: out u16 elem w <- in word w).
"""

import numpy as np
import ml_dtypes

D = 128          # directions (ES population)
O = 256          # out channels
K = 2304         # flattened reduction length
NT = 18          # k-tiles of 128 per direction
P = 128          # partitions
NCORES = 8
DPC = D // NCORES   # directions per core
NQ = DPC // 4       # quads per core
NBLK = 72           # derived 256-col blocks per quad
NPAIR = NBLK // 2   # DoubleRow pairs per quad
WPQ = 72 * O // 16  # packed u16 words per partition per quad = 1152

FP8 = ml_dtypes.float8_e4m3

# pass table: (kind, shift); bit positions extracted into byte h's 0x40:
#   'r': 6+s+8h   'l': 6-s+8h
PASSES = [("r", 0), ("r", 1), ("l", 1), ("l", 2), ("l", 3),
          ("l", 4), ("l", 5), ("l", 6)]
PASS_BITS = [(6, 14), (7, 15), (5, 13), (4, 12), (3, 11),
             (2, 10), (1, 9), (0, 8)]

# per-quad pass-region lengths in 256-col blocks. Quad 3 uses a short
# LAST region so the final derive op (which gates the epilogue) is tiny;
# its packed stream widens to max(len)*256 bytes (capacity 2 bits per
# covered word per pass; uncovered word-bits are zero-padded).
REG_LEN = [[9] * 8, [9] * 8, [9] * 8, [10, 10, 10, 10, 10, 10, 10, 2]]
REG_OFF = [np.concatenate(([0], np.cumsum(l)[:-1])).astype(int)
           for l in REG_LEN]
QWORDS = [max(l) * 128 for l in REG_LEN]       # u16 words per partition
QBYTES = [2 * w for w in QWORDS]               # packed bytes per partition
QOFFB = np.concatenate(([0], np.cumsum(QBYTES)[:-1])).astype(int)
WTOT = int(sum(QBYTES))                        # 9472 bytes/partition

# block -> (j, t) map
_B = np.arange(NBLK)
BLK_J = (_B & 3).astype(np.int64)
BLK_T = (_B >> 2).astype(np.int64)

_nc_cache = {}


def _emit(tc, res_ap, wT_ap, xc_ap, hdr_ap):
    """Emit the per-core program into TileContext tc."""
    import concourse.mybir as mybir

    nc = tc.nc
    fp8 = mybir.dt.float8e4
    u16 = mybir.dt.uint16
    f16 = mybir.dt.float16
    f32 = mybir.dt.float32
    u8 = mybir.dt.uint8
    AT = mybir.AluOpType

    with (
        tc.tile_pool(name="w", bufs=1) as wp,
        tc.tile_pool(name="small", bufs=1) as sp,
        tc.tile_pool(name="ps", bufs=1, space="PSUM") as pp,
    ):
        # x coefficient columns: xc[p, q*72 + B] = 0.5 * x[4q+j(B), 128*t(B)+p]
        xc = sp.tile([P, NQ * NBLK], fp8)
        # header: kf = floor(bias) [4, NQ*O] ++ selneg [4,128]
        hdr = sp.tile([4, NQ * O + P], f16)

        # quads 0 and 3 solo; quads 1+2 share one tile pair with their
        # packed words and derived pass-regions interleaved contiguously,
        # so each middle derive op covers both quads with plain 2D APs.
        pq0 = wp.tile([P, QBYTES[0]], u8, name="pq0")
        pqP = wp.tile([P, QBYTES[1] + QBYTES[2]], u8, name="pqP")
        pq3 = wp.tile([P, QBYTES[3]], u8, name="pq3")
        dt0 = wp.tile([P, NBLK * O], fp8, name="dt0")
        dtP = wp.tile([P, 2 * NBLK * O], fp8, name="dtP")
        dt3 = wp.tile([P, NBLK * O], fp8, name="dt3")
        # queue plan: chunk0 alone first on sync (earliest visibility);
        # small hdr/xc + chunk1 on scalar; chunk2 on sync, chunk3 on scalar.
        def chunk(q):
            return wT_ap[:, int(QOFFB[q]) : int(QOFFB[q]) + QBYTES[q]]

        nc.sync.dma_start(out=pq0[:], in_=chunk(0))
        nc.scalar.dma_start(out=hdr[:], in_=hdr_ap)
        nc.scalar.dma_start(out=xc[:], in_=xc_ap)
        nc.sync.dma_start(out=pqP[:, 0 : QBYTES[1]], in_=chunk(1))
        nc.scalar.dma_start(out=pqP[:, QBYTES[1] :], in_=chunk(2))
        nc.scalar.dma_start(out=pq3[:], in_=chunk(3))

        res_a = wp.tile([P, 3 * O], u8)
        res_b = wp.tile([P, O], u8)
        # per-quad PSUM tiles (separate tiles -> no false WAR deps between
        # one quad's compare and the next quad's matmuls)
        ps_q = [pp.tile([P, 2 * O], f32, tag=f"psq{q}", name=f"ps_q{q}")
                for q in range(NQ)]
        ps_dummy = pp.tile([P, 2 * O], f32)
        probe = sp.tile([1, 4], f32)

        # PE + DVE warm-up (HAM clock gate ramps on chip activity): dummy
        # matmuls and memsets before the first weight chunk lands.
        scratch = sp.tile([P, 2 * O], fp8)
        nc.vector.memset(scratch[:], 0.0)
        neg128 = sp.tile([P, 1], f32)
        nc.vector.memset(neg128[:], -128.0)
        for w in range(18):
            nc.tensor.matmul(
                ps_dummy[0:32, O : 2 * O],
                scratch[:, 0:32],
                scratch[:, 0:O],
                start=True,
                stop=True,
                tile_position=(0, 0),
                skip_group_check=True,
            )

        # ACT sigmoid table warm-up probe (reads the dummy window).
        nc.scalar.activation(
            out=probe[:], in_=ps_dummy[0:1, O : O + 4],
            func=mybir.ActivationFunctionType.Sigmoid, scale=256.0,
            bias=neg128[0:1, :],
        )

        def _ts(out, in0, pi):
            kind, s = PASSES[pi]
            op0 = AT.logical_shift_right if kind == "r" else AT.logical_shift_left
            if kind == "r" and s == 0:
                nc.vector.tensor_scalar(out=out, in0=in0, scalar1=0x4040,
                                        scalar2=None, op0=AT.bitwise_and)
            else:
                nc.vector.tensor_scalar(out=out, in0=in0, scalar1=s,
                                        scalar2=0x4040, op0=op0,
                                        op1=AT.bitwise_and)

        def derive(g, pi):
            # g: 0 -> quad0, 1 -> quads 1+2 fused, 2 -> quad3
            if g == 1:
                out = dtP[:, pi * 2 * 9 * O : (pi + 1) * 2 * 9 * O].bitcast(u16)
                in0 = pqP[:, :].bitcast(u16)
                _ts(out, in0, pi)
                return
            q = 0 if g == 0 else 3
            tile, pq = (dt0, pq0) if g == 0 else (dt3, pq3)
            off, ln = int(REG_OFF[q][pi]), REG_LEN[q][pi]
            _ts(tile[:, off * O : (off + ln) * O].bitcast(u16),
                pq[:, 0 : ln * 256].bitcast(u16), pi)

        def dt_block(q, B):
            if q == 0:
                return dt0[:, B * O : (B + 1) * O]
            if q == 3:
                return dt3[:, B * O : (B + 1) * O]
            pi, i = B // 9, B % 9
            base = pi * 2 * 9 * O + (q - 1) * 9 * O + i * O
            return dtP[:, base : base + O]

        def bias_mm(q):
            nc.tensor.matmul(
                ps_q[q][:, 0:O],
                hdr[0:4, NQ * O : NQ * O + P],
                hdr[0:4, q * O : (q + 1) * O],
                start=True,
                stop=False,
                skip_group_check=True,
            )

        def blk_mm(q, B):
            j = int(BLK_J[B])
            nc.tensor.matmul(
                ps_q[q][32 * j : 32 * (j + 1), 0:O],
                xc[:, q * NBLK + B : q * NBLK + B + 1].broadcast_to((P, 32)),
                dt_block(q, B),
                start=False,
                stop=(B >= NBLK - 4),
                tile_position=(0, 32 * j),
                skip_group_check=True,
            )

        def compare(q):
            out = res_b[:, 0:O] if q == 3 else res_a[:, q * O : (q + 1) * O]
            nc.scalar.activation(
                out=out,
                in_=ps_q[q][:, 0:O],
                func=mybir.ActivationFunctionType.Sigmoid,
                scale=256.0, bias=neg128[:],
            )

        for q in range(NQ):
            bias_mm(q)

        for pi in range(8):
            derive(0, pi)
            for B in range(9 * pi, 9 * pi + 9):
                blk_mm(0, B)
        compare(0)
        for pi in range(8):
            derive(1, pi)
            for B in range(9 * pi, 9 * pi + 9):
                blk_mm(1, B)
            for B in range(9 * pi, 9 * pi + 9):
                blk_mm(2, B)
        compare(1)
        compare(2)
        nc.sync.dma_start(out=res_ap[:, : 3 * O], in_=res_a[0:P:32, :])
        for pi in range(8):
            derive(2, pi)
            off, ln = int(REG_OFF[3][pi]), REG_LEN[3][pi]
            for B in range(off, off + ln):
                blk_mm(3, B)
        compare(3)
        nc.sync.dma_start(out=res_ap[:, 3 * O :], in_=res_b[0:P:32, :])


def _build():
    """Build the per-core Bass program (same NEFF on all 8 cores)."""
    import concourse.bacc as bacc
    import concourse.mybir as mybir
    from concourse.tile import TileContext

    nc = bacc.Bacc("TRN2", debug=False, enable_asserts=False)

    fp8 = mybir.dt.float8e4
    f16 = mybir.dt.float16
    u8 = mybir.dt.uint8

    wT = nc.dram_tensor("wT", [P, WTOT], u8, kind="ExternalInput")
    xc = nc.dram_tensor("xc", [P, NQ * NBLK], fp8, kind="ExternalInput")
    hdr = nc.dram_tensor("hdr", [4, NQ * O + P], f16, kind="ExternalInput")
    res = nc.dram_tensor("res", [4, NQ * O], u8, kind="ExternalOutput")

    with TileContext(nc) as tc:
        _emit(tc, res.ap(), wT.ap(), xc.ap(), hdr.ap())
    nc.compile()
    return nc


def prepare_inputs(weight_noise, bias_noise, x):
    """Host-side dtype cast + bit packing + layout transform + sharding.

    All transforms are data-independent (fixed index shuffles, the exact
    0/1 bit pack into u16 fields, fixed scalar prescale of the 0/1
    x-bits, and floor() of the compare constant); the reduction/compare
    math runs on device.
    """
    w = np.asarray(weight_noise).astype(np.uint8)       # [D, O, K] 0/1
    w4 = w.reshape(D, O, NT, P)                         # [d, o, t, p]
    xb = np.asarray(x).astype(np.float32)               # [D, K] 0/1

    kf = np.floor(np.asarray(bias_noise).astype(np.float64)).astype(np.float16)
    selneg = np.zeros((4, P), dtype=np.float16)
    for j in range(4):
        selneg[j, 32 * j : 32 * (j + 1)] = -1.0

    in_maps = []
    for c in range(NCORES):
        wq = np.zeros((P, WTOT), dtype=np.uint8)
        xcq = np.empty((P, NQ * NBLK), dtype=FP8)
        for q in range(NQ):
            dq = c * DPC + 4 * q
            # DB[p, B, o] = w[dq + j(B), o, t(B), p]
            DB = w4[dq + BLK_J, :, BLK_T, :]            # [72, O, P]
            DB = np.ascontiguousarray(DB.transpose(2, 0, 1))  # [P, 72, O]
            DBf = DB.reshape(P, NBLK * O)
            PQ = np.zeros((P, QWORDS[q]), dtype=np.uint16)
            for pi in range(8):
                n0, n1 = PASS_BITS[pi]
                off, ln = int(REG_OFF[q][pi]), REG_LEN[q][pi]
                L = ln * 128
                reg = DBf[:, off * O : (off + ln) * O].reshape(P, L, 2)
                PQ[:, :L] |= reg[:, :, 0].astype(np.uint16) << np.uint16(n0)
                PQ[:, :L] |= reg[:, :, 1].astype(np.uint16) << np.uint16(n1)
            wq[:, int(QOFFB[q]) : int(QOFFB[q]) + QBYTES[q]] = PQ.view(np.uint8)
            # xc[p, q*72 + B] = 0.5 * x[dq + j(B), 128*t(B) + p]
            xv = xb[dq + BLK_J, :].reshape(NBLK, NT, P)  # [B, t, p]
            xsel = xv[np.arange(NBLK), BLK_T, :]         # [B, p]
            xcq[:, q * NBLK : (q + 1) * NBLK] = (0.5 * xsel.T).astype(FP8)

        kc = (
            kf[c * DPC : (c + 1) * DPC]
            .reshape(NQ, 4, O)
            .transpose(1, 0, 2)
            .reshape(4, NQ * O)
        )
        hc = np.concatenate([kc, selneg], axis=1)
        in_maps.append(
            {
                "wT": wq,
                "xc": np.ascontiguousarray(xcq),
                "hdr": np.ascontiguousarray(hc),
            }
        )
    return in_maps


def core_output(res):
    """Decode one core's raw res buffer into [DPC, O] bool."""
    return (
        np.asarray(res).reshape(4, NQ, O).transpose(1, 0, 2).reshape(DPC, O)
    ).astype(bool)


def run(weight_noise, bias_noise, x, trace=False, **spmd_kwargs):
    """Run on the 8 NeuronCores; returns (bool [D, O] output, results)."""
    from concourse.bass_utils import run_bass_kernel_spmd

    in_maps = prepare_inputs(weight_noise, bias_noise, x)
    if "nc" in _nc_cache:
        nc = _nc_cache["nc"]
    else:
        nc = _nc_cache["nc"] = _build()
    r = run_bass_kernel_spmd(
        nc, in_maps, core_ids=list(range(NCORES)), trace=trace, **spmd_kwargs
    )
    out = np.concatenate(
        [core_output(r.results[c]["res"]) for c in range(NCORES)], axis=0
    )
    return out, r


def kernel(weight_noise, bias_noise, x):
    out, _ = run(weight_noise, bias_noise, x)
    return out
